# revision 7
# baseline (speedup 1.0000x reference)
"""GAT (graph-attention) layer on 8 Trainium2 NeuronCores.

Problem: B=8 graphs, N=2048 nodes, F=256 features.
    h   = x @ W                                  [B,N,F]
    s1  = h @ a1 ; s2 = h @ a2                   [B,N]
    e   = leaky_relu(s1[:,i,None] + s2[:,None,j], 0.2)
    att = softmax(where(adj>0, e, -9e15), axis=1)    # over i!
    out = elu(att @ h)

Sharding: data-parallel, one graph per NeuronCore (B=8, 8 cores).

Device algorithm (per core), all in natural layouts:
  - Host feeds xT = x.T (fp16), maskT[j,i] = 0/-240 additive mask (fp16,
    = transpose of adjacency), W (fp16), a as [2,256] (fp16).
  - w12 = W @ [a1,a2]        (DVE mul+reduce, tiny)
  - s12T = w12.T @ xT        -> s1 as a row  -> broadcast to s1b [128,N]
  - s12  = xT.T @ w12        -> s2 as per-partition columns
  - h    = xT.T @ W          [N,F] f32 (fp16 matmul)
  - per j-tile (16 tiles of 128 rows of the TRANSPOSED attention):
      u   = s1b + maskT_tile          (DVE fp16 2x)
      sl  = Lrelu(u + s2col)          (ACT, bias = per-partition s2)
      pT  = Exp(sl) -> bf16, accum_out = den (softmax denominator!)
      g   = h_tile * (1/den)          -> bf16
      hp[it] += pT[:, it-block].T @ g (PE, PSUM accumulates all 16 i-tiles)
  - epilogue: out = elu(hp) -> DMA out.

Softmax max-subtraction is skipped: scores are ~N(0, 8), exp stays in
f32/bf16 range; masked entries get -240 additive -> exp ~ 1e-20.
"""

import os
import sys

sys.path.insert(0, "/opt/trn_rl_repo")

import numpy as np

import concourse.bass as bass
import concourse.bacc as bacc
import concourse.tile as tile
from concourse import mybir
from concourse.bass_utils import run_bass_kernel_spmd

B, N, F = 8, 2048, 256
P = 128
NT = N // P        # 16 node tiles
FC = F // P        # 2 feature chunks
MASK_NEG = -240.0
ALPHA = 0.2

f32 = mybir.dt.float32
f16 = mybir.dt.float16
bf16 = mybir.dt.bfloat16

_CACHE = {}


def _build_nc():
    nc = bacc.Bacc(
        "TRN2",
        target_bir_lowering=False,
        debug=False,
        enable_asserts=False,
    )
    xT = nc.dram_tensor("xT", [F, N], f16, kind="ExternalInput")
    maskT = nc.dram_tensor("maskT", [N, N], f16, kind="ExternalInput")
    Wd = nc.dram_tensor("W", [F, F], f16, kind="ExternalInput")
    a2 = nc.dram_tensor("a2", [2, F], f16, kind="ExternalInput")
    out = nc.dram_tensor("out", [N, F], f32, kind="ExternalOutput")
    s1dram = nc.dram_tensor("s1row_dram", [1, N], f16)

    with tile.TileContext(nc) as tc:
        with tc.tile_pool(name="const", bufs=1) as cpool:
            # ---- constant loads -------------------------------------------------
            xT_sb = cpool.tile([P, FC, N], f16, tag="xT_sb")
            for fc in range(FC):
                nc.sync.dma_start(xT_sb[:, fc, :], xT[fc * P:(fc + 1) * P, :])
            W_sb = cpool.tile([P, FC, F], f16, tag="W_sb")
            for fc in range(FC):
                nc.sync.dma_start(W_sb[:, fc, :], Wd[fc * P:(fc + 1) * P, :])
            a_bc = cpool.tile([P, 2, F], f16, tag="a_bc")
            for m in range(2):
                nc.sync.dma_start(
                    a_bc[:, m, :], a2[m:m + 1, :].to_broadcast([P, F])
                )

            s1b = cpool.tile([P, N], f16, tag="s1b")
            s2cols = cpool.tile([P, NT], f32, tag="s2cols")
            h_sb = cpool.tile([P, NT, F], f32, tag="h_sb")
            w12_f32 = cpool.tile([P, FC, 2], f32, tag="w12f")
            w12 = cpool.tile([P, FC, 2], f16, tag="w12h")
            s1row = cpool.tile([1, N], f16, tag="s1row")

            # ---- phase 0: w12, s1/s2, h ----------------------------------------
            with tc.tile_pool(name="p0sb", bufs=2) as p0sb, tc.tile_pool(
                name="p0psum", bufs=2, space="PSUM"
            ) as p0ps:
                # w12[f, m] = sum_o W[f, o] * a_m[o]
                for fc in range(FC):
                    for m in range(2):
                        wtmp = p0sb.tile([P, F], f16, tag="wtmp")
                        nc.vector.tensor_mul(wtmp[:], W_sb[:, fc, :], a_bc[:, m, :])
                        nc.vector.tensor_reduce(
                            w12_f32[:, fc, m:m + 1], wtmp[:],
                            mybir.AxisListType.X, mybir.AluOpType.add,
                        )
                nc.vector.tensor_copy(w12[:], w12_f32[:])

                # s12T[2, N] = w12.T @ xT  (row 0 = s1, row 1 = s2)
                for c in range(4):
                    ps = p0ps.tile([2, 512], f32, tag="psT")
                    for fc in range(FC):
                        nc.tensor.matmul(
                            ps[:],
                            w12[:, fc, :],
                            xT_sb[:, fc, c * 512:(c + 1) * 512],
                            start=(fc == 0),
                            stop=(fc == FC - 1),
                        )
                    nc.scalar.copy(s1row[0:1, c * 512:(c + 1) * 512], ps[0:1, :])

                # s12 natural: s2 per-partition columns
                for jc in range(NT):
                    ps2 = p0ps.tile([P, 2], f32, tag="ps2")
                    for fc in range(FC):
                        nc.tensor.matmul(
                            ps2[:],
                            xT_sb[:, fc, jc * P:(jc + 1) * P],
                            w12[:, fc, :],
                            start=(fc == 0),
                            stop=(fc == FC - 1),
                        )
                    nc.scalar.copy(s2cols[:, jc:jc + 1], ps2[:, 1:2])

                # h = x @ W   (per j-chunk of 128 rows)
                for jc in range(NT):
                    ph = p0ps.tile([P, F], f32, tag="ph")
                    for fc in range(FC):
                        nc.tensor.matmul(
                            ph[:],
                            xT_sb[:, fc, jc * P:(jc + 1) * P],
                            W_sb[:, fc, :],
                            start=(fc == 0),
                            stop=(fc == FC - 1),
                        )
                    nc.scalar.copy(h_sb[:, jc, :], ph[:])

                # broadcast s1 row -> all partitions (via DRAM round-trip)
                nc.gpsimd.dma_start(s1dram[0:1, :], s1row[0:1, :])
                nc.sync.dma_start(s1b[:], s1dram[0:1, :].to_broadcast([P, N]))

            # ---- main loop over j-tiles ----------------------------------------
            with tc.tile_pool(name="hp", bufs=1, space="PSUM") as hppool, \
                 tc.tile_pool(name="loop", bufs=3) as lpool, \
                 tc.tile_pool(name="small", bufs=4) as spool, \
                 tc.tile_pool(name="ep", bufs=4) as epool:
                hp = [
                    hppool.tile([P, 512], f32, name=f"hp{b}", tag=f"hp{b}") for b in range(NT // 2)
                ]
                for jc in range(NT):
                    mk = lpool.tile([P, N], f16, tag="mk")
                    nc.sync.dma_start(mk[:], maskT[jc * P:(jc + 1) * P, :])
                    u = lpool.tile([P, N], f16, tag="u")
                    nc.vector.tensor_add(u[:], s1b[:], mk[:])
                    sl = lpool.tile([P, N], f16, tag="sl")
                    nc.scalar.activation(
                        sl[:], u[:], mybir.ActivationFunctionType.Prelu,
                        bias=s2cols[:, jc:jc + 1], scale=1.0, alpha=ALPHA,
                    )
                    pt = lpool.tile([P, N], bf16, tag="pt")
                    den = spool.tile([P, 1], f32, tag="den")
                    nc.scalar.activation(
                        pt[:], sl[:], mybir.ActivationFunctionType.Exp,
                        accum_out=den[:],
                    )
                    dinv = spool.tile([P, 1], f32, tag="dinv")
                    nc.vector.reciprocal(dinv[:], den[:])
                    g = spool.tile([P, F], bf16, tag="g")
                    nc.vector.tensor_scalar_mul(g[:], h_sb[:, jc, :], dinv[:])
                    for it in range(NT):
                        # NOTE: start=True zeroes the whole PSUM *bank*, so
                        # only the first region written into each bank may
                        # set it; the second region accumulates onto the
                        # bank-zeroed half.
                        nc.tensor.matmul(
                            hp[it // 2][:, (it % 2) * F:(it % 2 + 1) * F],
                            pt[:, it * P:(it + 1) * P],
                            g[:],
                            start=(jc == 0 and it % 2 == 0),
                            stop=(jc == NT - 1 and it % 2 == 1),
                        )

                # ---- epilogue: elu + store -------------------------------------
                for it in range(NT):
                    src = hp[it // 2][:, (it % 2) * F:(it % 2 + 1) * F]
                    e = epool.tile([P, F], f32, tag="e")
                    nc.scalar.activation(
                        e[:], src, mybir.ActivationFunctionType.Exp
                    )
                    r = epool.tile([P, F], f32, tag="r")
                    nc.scalar.activation(
                        r[:], src, mybir.ActivationFunctionType.Relu
                    )
                    nc.vector.tensor_scalar_add(e[:], e[:], -1.0)
                    nc.vector.copy_predicated(
                        e[:], r[:].bitcast(mybir.dt.uint32), r[:]
                    )
                    nc.gpsimd.dma_start(out[it * P:(it + 1) * P, :], e[:])

    nc.compile()
    return nc


def _get_nc():
    if "nc" not in _CACHE:
        _CACHE["nc"] = _build_nc()
    return _CACHE["nc"]


def _prep_inputs(x, adj, W, a):
    """Host-side sharding + layout prep: one graph per core."""
    W16 = np.ascontiguousarray(W.astype(np.float16))
    a2 = np.ascontiguousarray(a.reshape(2, F).astype(np.float16))
    in_maps = []
    for b in range(B):
        xT = np.ascontiguousarray(x[b].T.astype(np.float16))
        adjT = adj[b].T
        maskT = np.where(adjT > 0, np.float16(0.0), np.float16(MASK_NEG))
        in_maps.append(
            {"xT": xT, "maskT": np.ascontiguousarray(maskT),
             "W": W16, "a2": a2}
        )
    return in_maps


def run(x, adj, W, a, trace=False, **spmd_kwargs):
    nc = _get_nc()
    in_maps = _prep_inputs(x, adj, W, a)
    res = run_bass_kernel_spmd(
        nc, in_maps, core_ids=list(range(B)), trace=trace, **spmd_kwargs
    )
    outs = [np.asarray(r["out"], dtype=np.float32) for r in res.results]
    _CACHE["last_exec_ns"] = res.exec_time_ns
    _CACHE["last_result"] = res
    return np.stack(outs, axis=0)


def kernel(x, adj, W, a):
    x = np.asarray(x, dtype=np.float32)
    adj = np.asarray(adj)
    W = np.asarray(W, dtype=np.float32)
    a = np.asarray(a, dtype=np.float32)
    return run(x, adj, W, a, trace=False)
